# revision 1
# baseline (speedup 1.0000x reference)
"""Causal self-attention (B=2, S=2048, D=2048, H=16) on 8 trn2 NeuronCores.

Sharding: core c -> batch b = c//4, head-group hg = c%4 (4 heads of 128 dims).
Each core computes its heads' attention plus the partial output projection
(row-parallel split of W_proj); the host sums the 4 partials per batch.

All heavy matmuls run as float32r (full PE rate at free-dim >= 256); every
tensor feeding a matmul is produced as float32r end-to-end, which the BIR
verifier requires.
"""

import sys

sys.path.insert(0, "/opt/trn_rl_repo")

from contextlib import ExitStack

import numpy as np

import concourse.bass as bass
import concourse.mybir as mybir
import concourse.tile as tile
from concourse import bacc
from concourse.bass_utils import run_bass_kernel_spmd

B, S, D, H = 2, 2048, 2048, 16
HD = D // H  # 128
NH = 4  # heads per core
HG = H // NH  # head groups = 4
P = 128
KT = D // P  # 16 k-tiles over model dim
NQ = 4  # q-groups of 512
QW = S // NQ  # 512
ST = S // P  # 16 s-tiles of 128
SCALE = float(1.0 / np.sqrt(D).astype(np.float32))
MASK_NEG = -30000.0  # exp(SCALE * -30000) == 0 in fp32

F32 = mybir.dt.float32
F32R = mybir.dt.float32r


def build_bass():
    nc = bacc.Bacc("TRN2")

    xT = nc.declare_dram_parameter("xT", [D, S], F32R, isOutput=False)
    wq = nc.declare_dram_parameter("wq", [D, NH * HD], F32R, isOutput=False)
    wk = nc.declare_dram_parameter("wk", [D, NH * HD], F32R, isOutput=False)
    wv = nc.declare_dram_parameter("wv", [D, NH * HD], F32R, isOutput=False)
    wp = nc.declare_dram_parameter("wp", [NH * HD, D], F32R, isOutput=False)
    mask = nc.declare_dram_parameter("mask", [P, 3 * P], F32, isOutput=False)
    y = nc.declare_dram_parameter("y", [S, D], F32, isOutput=True)

    with tile.TileContext(nc) as tc, ExitStack() as top:
        const = top.enter_context(tc.tile_pool(name="const", bufs=1))
        dram = top.enter_context(tc.tile_pool(name="dram", bufs=1, space="DRAM"))

        # mask[:, 0:128]: triangular block mask (col >= row allowed);
        # mask[:, 128:384]: r=3 variant covering cols [256:512) of the
        # q-group (first 128 cols fully masked, last 128 triangular).
        mask_sb = const.tile([P, 3 * P], F32)
        nc.sync.dma_start(mask_sb, mask[:, :])
        ones_f32 = const.tile([P, 1], F32)
        nc.vector.memset(ones_f32, 1.0)
        ones_col = const.tile([P, 1], F32R)  # lhsT for denominator matmuls
        nc.vector.tensor_copy(ones_col, ones_f32)

        qt_scr = dram.tile([NH * HD, S], F32R)  # Q^T per head stacked
        kt_scr = dram.tile([NH * HD, S], F32R)  # K^T per head stacked
        v_scr = dram.tile([S, NH * HD], F32R)  # V natural layout

        # ---------------- Phase 1: QKV projections ----------------
        with ExitStack() as ph1:
            wpool = ph1.enter_context(tc.tile_pool(name="wpool", bufs=1))
            xpool = ph1.enter_context(tc.tile_pool(name="xpool", bufs=2))
            ppool = ph1.enter_context(tc.tile_pool(name="p1psum", bufs=4, space="PSUM"))
            bpool = ph1.enter_context(tc.tile_pool(name="p1bounce", bufs=4))

            # Load order matters: the first matmul group (n=0, h=0, wq)
            # accumulates over k-tiles in order, so k-chunked loads of
            # x-block 0 and the h=0 weight slices let PE start ~5us in
            # instead of waiting for whole tensors.
            xT_r = xT[:, :].rearrange("(k p) s -> p k s", p=P)
            wq_r = wq[:, :].rearrange("(k p) m -> p k m", p=P)
            wk_r = wk[:, :].rearrange("(k p) m -> p k m", p=P)
            xblk0 = xpool.tile([P, KT, QW], F32R, tag="xblk")
            wq_sb = wpool.tile([P, KT, NH * HD], F32R, tag="wq")
            wk_sb = wpool.tile([P, KT, NH * HD], F32R, tag="wk")
            c0s = slice(0, HD)
            for kc in range(0, KT, 4):
                ks = slice(kc, kc + 4)
                nc.sync.dma_start(xblk0[:, ks, :], xT_r[:, ks, 0:QW])
                nc.sync.dma_start(wq_sb[:, ks, c0s], wq_r[:, ks, c0s])
                nc.sync.dma_start(wk_sb[:, ks, c0s], wk_r[:, ks, c0s])
            for h in range(1, NH):
                cs = slice(h * HD, (h + 1) * HD)
                nc.sync.dma_start(wq_sb[:, :, cs], wq_r[:, :, cs])
                nc.sync.dma_start(wk_sb[:, :, cs], wk_r[:, :, cs])
            wv_sb = wpool.tile([P, KT, NH * HD], F32R, tag="wv")
            nc.sync.dma_start(wv_sb, wv[:, :].rearrange("(k p) m -> p k m", p=P))

            for n in range(NQ):
                if n == 0:
                    xblk = xblk0
                else:
                    xblk = xpool.tile([P, KT, QW], F32R, tag="xblk")
                    nc.sync.dma_start(xblk, xT_r[:, :, n * QW : (n + 1) * QW])
                for h in range(NH):
                    for w_sb, scr in ((wq_sb, qt_scr), (wk_sb, kt_scr)):
                        ps = ppool.tile([P, QW], F32, tag="ps")
                        for k in range(KT):
                            nc.tensor.matmul(
                                ps,
                                lhsT=w_sb[:, k, h * HD : (h + 1) * HD],
                                rhs=xblk[:, k, :],
                                start=(k == 0),
                                stop=(k == KT - 1),
                            )
                        bo = bpool.tile([P, QW], F32R, tag="bo")
                        nc.scalar.copy(bo, ps)
                        nc.sync.dma_start(
                            scr[h * HD : (h + 1) * HD, n * QW : (n + 1) * QW], bo
                        )
                for mi in range(4):
                    ps = ppool.tile([P, QW], F32, tag="ps")
                    for k in range(KT):
                        nc.tensor.matmul(
                            ps,
                            lhsT=xblk[:, k, mi * P : (mi + 1) * P],
                            rhs=wv_sb[:, k, :],
                            start=(k == 0),
                            stop=(k == KT - 1),
                        )
                    bo = bpool.tile([P, QW], F32R, tag="bo")
                    nc.scalar.copy(bo, ps)
                    m = n * 4 + mi
                    nc.sync.dma_start(v_scr[m * P : (m + 1) * P, :], bo)

        # ---------------- Phase 2: attention per (head, q-group) ----------------
        apool = top.enter_context(tc.tile_pool(name="apool", bufs=NH))
        wppool = top.enter_context(tc.tile_pool(name="wppool", bufs=1))
        wp_sb = wppool.tile([P, NH, D], F32R)

        a_tiles = []
        with ExitStack() as ph2:
            kvpool = ph2.enter_context(tc.tile_pool(name="kvpool", bufs=2))
            qpool = ph2.enter_context(tc.tile_pool(name="qpool", bufs=3))
            epool = ph2.enter_context(tc.tile_pool(name="epool", bufs=6))
            rpool = ph2.enter_context(tc.tile_pool(name="rpool", bufs=2))
            rbpool = ph2.enter_context(tc.tile_pool(name="rbpool", bufs=2))
            spool = ph2.enter_context(tc.tile_pool(name="spsum", bufs=4, space="PSUM"))
            upool = ph2.enter_context(tc.tile_pool(name="upsum", bufs=2, space="PSUM"))
            dpool = ph2.enter_context(tc.tile_pool(name="dpsum", bufs=2, space="PSUM"))

            v_scr_r = v_scr[:, :].rearrange("(k p) m -> p k m", p=P)
            for h in range(NH):
                if h == 1:
                    # W_proj is only needed in phase 3 — load it after the
                    # first head so it doesn't steal DMA bandwidth from the
                    # phase-1->2 transition.
                    nc.sync.dma_start(
                        wp_sb, wp[:, :].rearrange("(k p) n -> p k n", p=P)
                    )
                rs = slice(h * HD, (h + 1) * HD)
                # K^T and V stream in per q-group chunk (group qg only needs
                # k-tiles <= 4qg+3) on the scalar-engine HWDGE queue, so the
                # first scores don't wait for the full 1MB head load.
                ktsb = kvpool.tile([P, S], F32R, tag="kt")
                vsb = kvpool.tile([P, ST, HD], F32R, tag="v")
                a_h = apool.tile([P, S], F32R, tag="a", name=f"a_{h}")
                a_tiles.append(a_h)
                for qg in range(NQ):
                    qs = slice(qg * QW, (qg + 1) * QW)
                    nc.scalar.dma_start(ktsb[:, qs], kt_scr[rs, qs])
                    nc.scalar.dma_start(
                        vsb[:, 4 * qg : 4 * qg + 4, :],
                        v_scr_r[:, 4 * qg : 4 * qg + 4, rs],
                    )
                    qsb = qpool.tile([P, QW], F32R)
                    nc.sync.dma_start(qsb, qt_scr[rs, qs])
                    kmax = 4 * qg + 4
                    ups = upool.tile([P, QW], F32, tag="u")
                    dps = dpool.tile([1, QW], F32, tag="d")
                    for kt in range(kmax):
                        r = kt - 4 * qg
                        # live columns of this k-tile start at r*128; fp32r
                        # needs >=256 moving cols for full rate, so compute
                        # from c0 and mask the dead cols.
                        c0 = 0 if r < 0 else min(r * P, QW - 2 * P)
                        sps = spool.tile([P, QW], F32, tag="s")
                        nc.tensor.matmul(
                            sps[:, c0:],
                            lhsT=ktsb[:, kt * P : (kt + 1) * P],
                            rhs=qsb[:, c0:],
                            start=True,
                            stop=True,
                        )
                        if r >= 0:
                            if r == 3:
                                nc.vector.tensor_tensor(
                                    sps[:, 2 * P :], sps[:, 2 * P :],
                                    mask_sb[:, P : 3 * P],
                                    op=mybir.AluOpType.add,
                                )
                            else:
                                nc.vector.tensor_tensor(
                                    sps[:, r * P : (r + 1) * P],
                                    sps[:, r * P : (r + 1) * P],
                                    mask_sb[:, 0:P],
                                    op=mybir.AluOpType.add,
                                )
                        e = epool.tile([P, QW], F32R, tag="e")
                        nc.scalar.activation(
                            e[:, c0:], sps[:, c0:],
                            mybir.ActivationFunctionType.Exp, scale=SCALE,
                        )
                        nc.tensor.matmul(
                            ups[:, c0:],
                            lhsT=vsb[:, kt, :],
                            rhs=e[:, c0:],
                            start=(kt == 0),
                            stop=(kt == kmax - 1),
                        )
                        nc.tensor.matmul(
                            dps[:, c0:],
                            lhsT=ones_col,
                            rhs=e[:, c0:],
                            start=(kt == 0),
                            stop=(kt == kmax - 1),
                        )
                    rcp = rpool.tile([1, QW], F32)
                    nc.vector.reciprocal_approx_fast(rcp, dps)
                    rb = rbpool.tile([P, QW], F32)
                    nc.gpsimd.partition_broadcast(rb, rcp)
                    nc.vector.tensor_tensor(
                        a_h[:, qs], ups, rb, op=mybir.AluOpType.mult
                    )

        # ---------------- Phase 3: output projection ----------------
        with ExitStack() as ph3:
            ypool = ph3.enter_context(tc.tile_pool(name="ypsum", bufs=4, space="PSUM"))
            ybp = ph3.enter_context(tc.tile_pool(name="ybounce", bufs=4))
            for m in range(ST):
                for n in range(NQ):
                    yp = ypool.tile([P, QW], F32, tag="yp")
                    for k in range(NH):
                        nc.tensor.matmul(
                            yp,
                            lhsT=a_tiles[k][:, m * P : (m + 1) * P],
                            rhs=wp_sb[:, k, n * QW : (n + 1) * QW],
                            start=(k == 0),
                            stop=(k == NH - 1),
                        )
                    yb = ybp.tile([P, QW], F32, tag="yb")
                    nc.scalar.copy(yb, yp)
                    nc.sync.dma_start(
                        y[m * P : (m + 1) * P, n * QW : (n + 1) * QW], yb
                    )

    nc.finalize()
    return nc


def _build_mask():
    # [:, 0:128]   triangular block mask: allowed iff col >= row
    # [:, 128:256] all masked (r=3 variant, cols [256:384) of the q-group)
    # [:, 256:384] triangular      (r=3 variant, cols [384:512))
    # Applied pre-scale: exp(SCALE * (score + mask)).
    k = np.arange(P)[:, None]
    c = np.arange(P)[None, :]
    tri = np.where(c >= k, 0.0, MASK_NEG).astype(np.float32)
    full = np.full((P, P), MASK_NEG, dtype=np.float32)
    return np.concatenate([tri, full, tri], axis=1)


_NC_CACHE = {}


def _get_nc():
    if "nc" not in _NC_CACHE:
        _NC_CACHE["nc"] = build_bass()
    return _NC_CACHE["nc"]


def make_in_maps(x, W_qkv, W_proj):
    x = np.asarray(x, dtype=np.float32)
    W_qkv = np.asarray(W_qkv, dtype=np.float32)
    W_proj = np.asarray(W_proj, dtype=np.float32)
    Wq, Wk, Wv = W_qkv[0:D], W_qkv[D : 2 * D], W_qkv[2 * D : 3 * D]
    mask = _build_mask()
    in_maps = []
    for c in range(8):
        b, hg = c // HG, c % HG
        rows = slice(hg * NH * HD, (hg + 1) * NH * HD)
        in_maps.append(
            {
                "xT": np.ascontiguousarray(x[b].T),
                "wq": np.ascontiguousarray(Wq[rows].T),
                "wk": np.ascontiguousarray(Wk[rows].T),
                "wv": np.ascontiguousarray(Wv[rows].T),
                "wp": np.ascontiguousarray(W_proj[:, rows].T),
                "mask": mask,
            }
        )
    return in_maps


def run(x, W_qkv, W_proj, trace=False):
    nc = _get_nc()
    in_maps = make_in_maps(x, W_qkv, W_proj)
    res = run_bass_kernel_spmd(nc, in_maps, core_ids=list(range(8)), trace=trace)
    out = np.zeros((B, S, D), dtype=np.float32)
    for c in range(8):
        out[c // HG] += res.results[c]["y"]
    return out, res


def kernel(x, W_qkv, W_proj):
    out, _ = run(x, W_qkv, W_proj, trace=False)
    return out



# revision 2
# speedup vs baseline: 1.4133x; 1.4133x over previous
"""Causal self-attention (B=2, S=2048, D=2048, H=16) on 8 trn2 NeuronCores.

Sharding: core c -> batch b = c//4, head-group hg = c%4 (4 heads of 128 dims).
Each core computes its heads' attention plus the partial output projection
(row-parallel split of W_proj); the host sums the 4 partials per batch.

Mixed precision built around fp8e4 DoubleRow matmuls (2 fp8 weights per PE
cell -> 256-deep contraction at 0.5 cycles/row):
- Q/K/V projections: fp8 DR (weights prescaled x64, evac copy scales 1/64)
- scores: bf16 (same PE rate as fp32r, half the SBUF traffic)
- PV + softmax denominators: fp8 DR over e8 = exp tiles written as fp8
- output projection: fp8 DR (a prescaled x16, evac scales 1/1024)
Early rows are precision-critical (softmax concentrates on few keys), so
queries 0-127 (which only see keys 0-127) run a bf16 path end-to-end:
bf16 V for keys 0-127, bf16 e/ups/denominator, and a bf16 m-tile-0 output
projection. Everything stays in SBUF; no DRAM scratch round trips.
"""

import sys

sys.path.insert(0, "/opt/trn_rl_repo")

from contextlib import ExitStack

import numpy as np
import ml_dtypes

import concourse.bass as bass
import concourse.mybir as mybir
import concourse.tile as tile
from concourse import bacc
from concourse.bass_utils import run_bass_kernel_spmd

B, S, D, H = 2, 2048, 2048, 16
HD = D // H  # 128
NH = 4  # heads per core
HG = H // NH  # head groups = 4
P = 128
KT = D // P  # 16 k-tiles over model dim
NQ = 4  # q-groups of 512
QW = S // NQ  # 512
ST = S // P  # 16 token-tiles of 128
SCALE = float(1.0 / np.sqrt(D).astype(np.float32))
MASK_NEG = -30000.0  # exp(SCALE * -30000) == 0 in fp32
WS = 64.0  # weight prescale for fp8
AS = 16.0  # attention-out prescale for fp8

F32 = mybir.dt.float32
BF = mybir.dt.bfloat16
F8 = mybir.dt.float8e4
DR = mybir.MatmulPerfMode.DoubleRow
ADD = mybir.AluOpType.add
MULT = mybir.AluOpType.mult
EXP = mybir.ActivationFunctionType.Exp
COPY = mybir.ActivationFunctionType.Copy

F8NP = ml_dtypes.float8_e4m3
BFNP = ml_dtypes.bfloat16


def build_bass():
    nc = bacc.Bacc("TRN2")

    x8 = nc.declare_dram_parameter("x8", [P, KT, S], F8, isOutput=False)
    wq8 = nc.declare_dram_parameter("wq8", [P, KT, NH * HD], F8, isOutput=False)
    wk8 = nc.declare_dram_parameter("wk8", [P, KT, NH * HD], F8, isOutput=False)
    wv8 = nc.declare_dram_parameter("wv8", [P, KT, NH * HD], F8, isOutput=False)
    wv16 = nc.declare_dram_parameter("wv16", [P, KT, NH * HD], BF, isOutput=False)
    xbT16 = nc.declare_dram_parameter("xbT16", [P, KT, P], BF, isOutput=False)
    wp8 = nc.declare_dram_parameter("wp8", [P, NH, D], F8, isOutput=False)
    wp16 = nc.declare_dram_parameter("wp16", [P, NH, D], BF, isOutput=False)
    mask = nc.declare_dram_parameter("mask", [P, 3 * P], F32, isOutput=False)
    y = nc.declare_dram_parameter("y", [S, D], F32, isOutput=True)

    with tile.TileContext(nc) as tc, ExitStack() as top:
        const = top.enter_context(tc.tile_pool(name="const", bufs=1))
        main = top.enter_context(tc.tile_pool(name="main", bufs=1))

        # ---- constants ----
        mask_sb = const.tile([P, 3 * P], F32)
        nc.gpsimd.dma_start(mask_sb, mask[:, :])
        ones8_t = const.tile([P, 2, 16], F8)
        nc.vector.memset(ones8_t, 1.0)
        ones8 = ones8_t[:, :, 0:1]  # DR lhsT: pair step 16B (ISA: step%16==0)
        ones16_t = const.tile([P, 16], BF)
        nc.vector.memset(ones16_t, 1.0)
        ones16 = ones16_t[:, 0:1]

        # ---- persistent tensors ----
        x8_sb = main.tile([P, KT, S], F8)
        wq8_sb = main.tile([P, KT, NH * HD], F8)
        wk8_sb = main.tile([P, KT, NH * HD], F8)
        wv8_sb = main.tile([P, KT, NH * HD], F8)
        wp8_sb = main.tile([P, NH, D], F8)
        wp16_sb = main.tile([P, NH, D], BF)
        qT = main.tile([P, NH, S], BF)
        kT = main.tile([P, NH, S], BF)
        v8 = main.tile([P, KT, NH * HD], F8)
        v16 = main.tile([P, NH * HD], BF)
        a8 = main.tile([P, NH, S], F8)
        a16 = main.tile([P, NH, P], BF)

        # ---- input DMAs, ordered by first use across 3 queues ----
        # sync queue: x (biggest, needed first)
        for kk in range(KT // 2):
            nc.sync.dma_start(
                x8_sb[:, 2 * kk : 2 * kk + 2, :], x8[:, 2 * kk : 2 * kk + 2, :]
            )
        # scalar queue: weights in first-use order
        for h in range(2):
            cs = slice(h * HD, (h + 1) * HD)
            nc.scalar.dma_start(wk8_sb[:, :, cs], wk8[:, :, cs])
            nc.scalar.dma_start(wq8_sb[:, :, cs], wq8[:, :, cs])
        nc.scalar.dma_start(wv8_sb, wv8[:, :, :])
        for h in range(2, NH):
            cs = slice(h * HD, (h + 1) * HD)
            nc.scalar.dma_start(wk8_sb[:, :, cs], wk8[:, :, cs])
            nc.scalar.dma_start(wq8_sb[:, :, cs], wq8[:, :, cs])
        nc.scalar.dma_start(wp8_sb, wp8[:, :, :])
        nc.scalar.dma_start(wp16_sb, wp16[:, :, :])

        # ---- psum pools (16KB/partition budget: 8+4+2+2) ----
        pbig = top.enter_context(tc.tile_pool(name="pbig", bufs=2, space="PSUM"))
        psmall = top.enter_context(tc.tile_pool(name="psmall", bufs=2, space="PSUM"))
        upool = top.enter_context(tc.tile_pool(name="upool", bufs=1, space="PSUM"))
        dpool = top.enter_context(tc.tile_pool(name="dpool", bufs=1, space="PSUM"))

        def kq_unit(h, w_sb, out_sb, evac_engine):
            """Head h of the Q or K projection: out[hd, tok] in bf16 (scale 1/WS).

            W-stationary fp8 DR, 2 token-tiles per weight load so LDWEIGHTS
            stays off the critical path.
            """
            cs = slice(h * HD, (h + 1) * HD)
            for npass in range(2):
                ps = [psmall.tile([P, QW], F32, tag="ps", name=f"kqp{j}") for j in range(2)]
                for kk in range(KT // 2):
                    for j in range(2):
                        n = 2 * npass + j
                        nc.tensor.matmul(
                            ps[j],
                            lhsT=w_sb[:, 2 * kk : 2 * kk + 2, cs],
                            rhs=x8_sb[:, 2 * kk : 2 * kk + 2, n * QW : (n + 1) * QW],
                            start=(kk == 0),
                            stop=(kk == KT // 2 - 1),
                            perf_mode=DR,
                        )
                for j in range(2):
                    n = 2 * npass + j
                    evac_engine(out_sb[:, h, n * QW : (n + 1) * QW], ps[j])

        def act_evac_ws(out, ps):
            nc.scalar.activation(out, ps, COPY, scale=1.0 / WS)

        def dve_evac_ws(out, ps):
            nc.vector.tensor_scalar_mul(out, ps, 1.0 / WS)

        def v_unit(m):
            """Token-tile m of the V projection -> v8[:, m, :] fp8 (scale 1/WS)."""
            ps = psmall.tile([P, QW], F32, tag="ps")
            for kk in range(KT // 2):
                nc.tensor.matmul(
                    ps,
                    lhsT=x8_sb[:, 2 * kk : 2 * kk + 2, m * P : (m + 1) * P],
                    rhs=wv8_sb[:, 2 * kk : 2 * kk + 2, :],
                    start=(kk == 0),
                    stop=(kk == KT // 2 - 1),
                    perf_mode=DR,
                )
            nc.vector.tensor_scalar_mul(v8[:, m, :], ps, 1.0 / WS)

        epool = top.enter_context(tc.tile_pool(name="epool", bufs=3))
        e16pool = top.enter_context(tc.tile_pool(name="e16pool", bufs=2))
        ypool = top.enter_context(tc.tile_pool(name="ypool", bufs=2))
        rpool = top.enter_context(tc.tile_pool(name="rpool", bufs=2))
        rbpool = top.enter_context(tc.tile_pool(name="rbpool", bufs=2))

        def s_unit(h, qg):
            """Scores + exp for (head, q-group). Returns (e8 tile, e16 tile)."""
            e8t = epool.tile([P, KT, QW], F8, tag="e8", name=f"e8_{h}_{qg}")
            e16t = None
            npairs = 2 * qg + 2
            qs0 = qg * QW
            for t in range(npairs):
                diag1 = t == 2 * qg + 1
                c0 = 2 * P if diag1 else 0
                F = QW - c0
                sp = pbig.tile([P, 2, QW], F32, tag="sp", name=f"sp{h}{qg}{t}")
                for i in range(2):
                    kt = 2 * t + i
                    nc.tensor.matmul(
                        sp[:, i, 0:F],
                        lhsT=kT[:, h, kt * P : (kt + 1) * P],
                        rhs=qT[:, h, qs0 + c0 : qs0 + QW],
                        start=True,
                        stop=True,
                    )
                if t >= 2 * qg:
                    # diagonal pair: first kt gets a triangular mask block,
                    # second kt gets [full | triangular]
                    nc.vector.tensor_tensor(
                        sp[:, 0, 0:P], sp[:, 0, 0:P], mask_sb[:, 0:P], op=ADD
                    )
                    nc.vector.tensor_tensor(
                        sp[:, 1, 0 : 2 * P], sp[:, 1, 0 : 2 * P],
                        mask_sb[:, P : 3 * P], op=ADD,
                    )
                nc.scalar.activation(
                    e8t[:, 2 * t : 2 * t + 2, c0:QW], sp[:, :, 0:F], EXP, scale=SCALE
                )
                if qg == 0 and t == 0:
                    e16t = e16pool.tile([P, P], BF, tag="e16")
                    nc.scalar.activation(e16t, sp[:, 0, 0:P], EXP, scale=SCALE)
            return e8t, e16t

        def pv_unit(h, qg, e8t, e16t):
            """PV + denominators + normalize for (head, q-group) -> a8 / a16."""
            cs = slice(h * HD, (h + 1) * HD)
            npairs = 2 * qg + 2
            qs0 = qg * QW
            up = upool.tile([P, QW], F32, tag="up")
            for t in range(npairs):
                c0 = 2 * P if t == 2 * qg + 1 else 0
                nc.tensor.matmul(
                    up[:, c0:],
                    lhsT=v8[:, 2 * t : 2 * t + 2, cs],
                    rhs=e8t[:, 2 * t : 2 * t + 2, c0:],
                    start=(t == 0),
                    stop=(t == npairs - 1),
                    perf_mode=DR,
                )
            dp = dpool.tile([1, QW], F32, tag="dp")
            for t in range(npairs):
                c0 = 2 * P if t == 2 * qg + 1 else 0
                nc.tensor.matmul(
                    dp[:, c0:],
                    lhsT=ones8,
                    rhs=e8t[:, 2 * t : 2 * t + 2, c0:],
                    start=(t == 0),
                    stop=(t == npairs - 1),
                    perf_mode=DR,
                )
            upe = None
            if qg == 0:
                # queries 0-127 attend only keys 0-127: bf16 numerator and
                # denominator (overwrite fp8 columns 0-127 of dp)
                upe = psmall.tile([P, QW], F32, tag="ps", name=f"upe{h}")
                nc.tensor.matmul(
                    upe[:, 0:P], lhsT=v16[:, cs], rhs=e16t, start=True, stop=True
                )
                nc.tensor.matmul(
                    dp[:, 0:P], lhsT=ones16, rhs=e16t, start=True, stop=True
                )
            rcp = rpool.tile([1, QW], F32, tag="rcp")
            nc.vector.reciprocal_approx_fast(rcp, dp)
            rb = rbpool.tile([P, QW], F32, tag="rb")
            nc.gpsimd.partition_broadcast(rb, rcp)
            lo = P if qg == 0 else 0
            nc.vector.scalar_tensor_tensor(
                a8[:, h, qs0 + lo : qs0 + QW],
                up[:, lo:], AS, rb[:, lo:], op0=MULT, op1=MULT,
            )
            if qg == 0:
                nc.vector.tensor_tensor(
                    a16[:, h, :], upe[:, 0:P], rb[:, 0:P], op=MULT
                )

        def c_unit(qg):
            """Output projection + DMA for the 4 token-tiles of q-group qg."""
            for m in range(4 * qg, 4 * qg + 4):
                y_sb = ypool.tile([P, S], F32, tag="ysb", name=f"ysb{m}")
                if m == 0:
                    # bf16 path for tokens 0-127
                    for npb in range(2):
                        ps = pbig.tile([P, 2, QW], F32, tag="sp", name=f"ym0{npb}")
                        for h in range(NH):
                            for nl in range(2):
                                n = 2 * npb + nl
                                nc.tensor.matmul(
                                    ps[:, nl, :],
                                    lhsT=a16[:, h, :],
                                    rhs=wp16_sb[:, h, n * QW : (n + 1) * QW],
                                    start=(h == 0),
                                    stop=(h == NH - 1),
                                )
                        for nl in range(2):
                            n = 2 * npb + nl
                            nc.vector.tensor_copy(
                                y_sb[:, n * QW : (n + 1) * QW], ps[:, nl, :]
                            )
                else:
                    ms = slice(m * P, (m + 1) * P)
                    pss = [
                        pbig.tile([P, 2, QW], F32, tag="sp", name=f"yp{m}{j}")
                        for j in range(2)
                    ]
                    for hp in range(2):
                        for npb in range(2):
                            for nl in range(2):
                                n = 2 * npb + nl
                                nc.tensor.matmul(
                                    pss[npb][:, nl, :],
                                    lhsT=a8[:, 2 * hp : 2 * hp + 2, ms],
                                    rhs=wp8_sb[:, 2 * hp : 2 * hp + 2, n * QW : (n + 1) * QW],
                                    start=(hp == 0),
                                    stop=(hp == 1),
                                    perf_mode=DR,
                                )
                    for npb in range(2):
                        for nl in range(2):
                            n = 2 * npb + nl
                            ev = nc.vector if nl == 0 else nc.scalar
                            if nl == 0:
                                nc.vector.tensor_scalar_mul(
                                    y_sb[:, n * QW : (n + 1) * QW],
                                    pss[npb][:, nl, :], 1.0 / (WS * AS),
                                )
                            else:
                                nc.scalar.activation(
                                    y_sb[:, n * QW : (n + 1) * QW],
                                    pss[npb][:, nl, :], COPY, scale=1.0 / (WS * AS),
                                )
                nc.sync.dma_start(y[m * P : (m + 1) * P, :], y_sb)

        # ---------------- main sequence ----------------
        with ExitStack() as pre:
            prepool = pre.enter_context(tc.tile_pool(name="prepool", bufs=1))
            wv16_sb = prepool.tile([P, KT, NH * HD], BF)
            xbT16_sb = prepool.tile([P, KT, P], BF)
            nc.gpsimd.dma_start(xbT16_sb, xbT16[:, :, :])
            nc.gpsimd.dma_start(wv16_sb, wv16[:, :, :])

            kq_unit(0, wk8_sb, kT, act_evac_ws)
            kq_unit(0, wq8_sb, qT, act_evac_ws)
            kq_unit(1, wk8_sb, kT, act_evac_ws)
            kq_unit(1, wq8_sb, qT, act_evac_ws)

            # bf16 V for keys 0-127
            psv = psmall.tile([P, QW], F32, tag="ps", name="psv16")
            for k in range(KT):
                nc.tensor.matmul(
                    psv,
                    lhsT=xbT16_sb[:, k, :],
                    rhs=wv16_sb[:, k, :],
                    start=(k == 0),
                    stop=(k == KT - 1),
                )
            nc.vector.tensor_copy(v16, psv)

        for qg in range(NQ):
            e_tiles = {}
            if qg == 0:
                e_tiles[0] = s_unit(0, 0)
                e_tiles[1] = s_unit(1, 0)
                kq_unit(2, wk8_sb, kT, dve_evac_ws)
                kq_unit(2, wq8_sb, qT, dve_evac_ws)
                for m in range(4):
                    v_unit(m)
                pv_unit(0, 0, *e_tiles[0])
                e_tiles[2] = s_unit(2, 0)
                kq_unit(3, wk8_sb, kT, dve_evac_ws)
                kq_unit(3, wq8_sb, qT, dve_evac_ws)
                pv_unit(1, 0, *e_tiles[1])
                e_tiles[3] = s_unit(3, 0)
                pv_unit(2, 0, *e_tiles[2])
                pv_unit(3, 0, *e_tiles[3])
            else:
                e_tiles[0] = s_unit(0, qg)
                e_tiles[1] = s_unit(1, qg)
                for m in range(4 * qg, 4 * qg + 4):
                    v_unit(m)
                pv_unit(0, qg, *e_tiles[0])
                e_tiles[2] = s_unit(2, qg)
                pv_unit(1, qg, *e_tiles[1])
                e_tiles[3] = s_unit(3, qg)
                pv_unit(2, qg, *e_tiles[2])
                pv_unit(3, qg, *e_tiles[3])
            c_unit(qg)

    nc.finalize()
    return nc


def _build_mask():
    # [tri | full | tri]: tri[p, c] = 0 where c >= p else MASK_NEG.
    # Applied pre-scale: exp(SCALE * (score + mask)).
    k = np.arange(P)[:, None]
    c = np.arange(P)[None, :]
    tri = np.where(c >= k, 0.0, MASK_NEG).astype(np.float32)
    full = np.full((P, P), MASK_NEG, dtype=np.float32)
    return np.concatenate([tri, full, tri], axis=1)


def _f8(a):
    return np.clip(a, -240.0, 240.0).astype(F8NP)


def _bf(a):
    return a.astype(BFNP)


def _pack_kps(mat_t, groups):
    """[rows=groups*128, cols] -> [128, groups, cols] with row = g*128+p."""
    r, c = mat_t.shape
    return np.ascontiguousarray(mat_t.reshape(groups, P, c).transpose(1, 0, 2))


_NC_CACHE = {}


def _get_nc():
    if "nc" not in _NC_CACHE:
        _NC_CACHE["nc"] = build_bass()
    return _NC_CACHE["nc"]


def make_in_maps(x, W_qkv, W_proj):
    x = np.asarray(x, dtype=np.float32)
    W_qkv = np.asarray(W_qkv, dtype=np.float32)
    W_proj = np.asarray(W_proj, dtype=np.float32)
    Wq, Wk, Wv = W_qkv[0:D], W_qkv[D : 2 * D], W_qkv[2 * D : 3 * D]
    mask = _build_mask()

    xb8 = []
    xbT = []
    for b in range(B):
        xT = x[b].T  # [D, S]
        xb8.append(_f8(_pack_kps(xT, KT)))
        xbT.append(_bf(_pack_kps(np.ascontiguousarray(xT[:, 0:P]), KT)))

    per_hg = []
    for hg in range(HG):
        rows = slice(hg * NH * HD, (hg + 1) * NH * HD)
        wq_t = Wq[rows].T  # [D, 512]
        wk_t = Wk[rows].T
        wv_t = Wv[rows].T
        wp_t = W_proj[:, rows].T  # [512, D]
        per_hg.append(
            {
                "wq8": _f8(_pack_kps(wq_t * WS, KT)),
                "wk8": _f8(_pack_kps(wk_t * WS, KT)),
                "wv8": _f8(_pack_kps(wv_t * WS, KT)),
                "wv16": _bf(_pack_kps(wv_t, KT)),
                "wp8": _f8(_pack_kps(wp_t * WS, NH)),
                "wp16": _bf(_pack_kps(wp_t, NH)),
            }
        )

    in_maps = []
    for c in range(8):
        b, hg = c // HG, c % HG
        m = {"x8": xb8[b], "xbT16": xbT[b], "mask": mask}
        m.update(per_hg[hg])
        in_maps.append(m)
    return in_maps


def run(x, W_qkv, W_proj, trace=False):
    nc = _get_nc()
    in_maps = make_in_maps(x, W_qkv, W_proj)
    res = run_bass_kernel_spmd(nc, in_maps, core_ids=list(range(8)), trace=trace)
    out = np.zeros((B, S, D), dtype=np.float32)
    for c in range(8):
        out[c // HG] += res.results[c]["y"]
    return out, res


def kernel(x, W_qkv, W_proj):
    out, _ = run(x, W_qkv, W_proj, trace=False)
    return out
